# revision 1
# baseline (speedup 1.0000x reference)
"""Self-contained Trainium2 kernel for nn_AutoregressiveGroupQuerySelfAttention.

Reference computation (B=2, S=2048, H=2048, 16 heads x 128 dim):
    q = (x @ Wq.T) -> heads; k likewise; v likewise
    q, k get RoPE; scores = (q @ k.T) * sqrt(D)   (faithful-to-source bug)
    causal softmax; ctx = attn @ v; out = ctx @ Wo.T

Sharding over 8 NeuronCores: core c = (b, g) with b = c // 4 (batch),
g = c % 4 (head-group of 4 heads = 512 hidden columns).  Each core computes
its head-group's context and a partial output  ctx_g @ Wo.T[g-rows, :];
the host sums the 4 partials per batch element.

Precision: the softmax is nearly one-hot (the sqrt(D) score scaling makes
logits ~N(0,128^2)), so the logit path (q/k projections + scores) runs in
fp32r (full-speed reduced-precision fp32 matmul, ~1.5e-4 rel err); v/P/Wo
run in bf16.  Measured end-to-end rel err vs the fp32 reference ~5e-3.

Schedule (per core):
  era 1 - q/k projections as kt-outer "waves" of 4 parallel PSUM chains
  (one per head), weights/x streamed in groups on the two HWDGE queues,
  RoPE finish deferred one wave so the PE never waits on Vector/Scalar.
  era 2 - c-outer/h-inner attention: per 512-row chunk, scores+softmax of
  heads 0-1 cover the v-projection DMAs, each head's P^T@V is deferred one
  head behind its scores, P transposes drain 4-at-a-time from a shared
  PSUM bank via one wide strided copy, and the output projection of chunk
  c-1 executes inside chunk c's first scores block to hide softmax latency.
"""
import numpy as np
import ml_dtypes

import concourse.bass as bass
import concourse.mybir as mybir
from concourse import bacc
from concourse.tile import TileContext
from concourse.bass_utils import run_bass_kernel_spmd

F32 = mybir.dt.float32
F32R = mybir.dt.float32r
BF16 = mybir.dt.bfloat16
AX = mybir.AxisListType
ALU = mybir.AluOpType
ACTF = mybir.ActivationFunctionType

B, S, H = 2, 2048, 2048
NUM_HEADS, D = 16, 128
N_CORES = 8
NH = 4                     # heads per core
HG = NH * D                # 512
ROPE_BASE = 10000.0

_NC_CACHE = {}
LAST_RESULTS = None        # BassKernelResults of the most recent run (for profiling)
TRACE = False


def _build(S_=S, H_=H, NH_=NH):
    DD = 128
    HG_ = NH_ * DD
    KT = H_ // 128
    SQT = S_ // 128
    CH = 512
    NCHUNK = S_ // CH

    nc = bacc.Bacc()
    xT = nc.declare_dram_parameter("xT", [H_, S_], F32R, isOutput=False)
    xbfT = nc.declare_dram_parameter("xbfT", [H_, S_], BF16, isOutput=False)
    wqT = nc.declare_dram_parameter("wqT", [H_, HG_], F32R, isOutput=False)
    wkT = nc.declare_dram_parameter("wkT", [H_, HG_], F32R, isOutput=False)
    wvT = nc.declare_dram_parameter("wvT", [H_, HG_], BF16, isOutput=False)
    woT = nc.declare_dram_parameter("woT", [HG_, H_], BF16, isOutput=False)
    cosT = nc.declare_dram_parameter("cosT", [128, S_], F32, isOutput=False)
    sinT = nc.declare_dram_parameter("sinT", [128, S_], F32, isOutput=False)
    rT = nc.declare_dram_parameter("rT", [128, 128], F32R, isOutput=False)
    ident = nc.declare_dram_parameter("ident", [128, 128], BF16, isOutput=False)
    identf = nc.declare_dram_parameter("identf", [128, 128], F32, isOutput=False)
    onesr = nc.declare_dram_parameter("onesr", [1, 128], BF16, isOutput=False)
    mask = nc.declare_dram_parameter("mask", [128, 128], F32, isOutput=False)
    out = nc.declare_dram_parameter("out", [S_, H_], F32, isOutput=True)

    with TileContext(nc) as tc:
        with (
            tc.tile_pool(name="slabs", bufs=1) as slabp,
            tc.tile_pool(name="stats", bufs=3) as statp,
            tc.tile_pool(name="rows", bufs=2) as rowp,
        ):
            qrope = [slabp.tile([128, S_], F32R, tag=f"qrope{h}", name=f"qrope{h}") for h in range(NH_)]
            krope = [slabp.tile([128, S_], F32R, tag=f"krope{h}", name=f"krope{h}") for h in range(NH_)]
            vslab = slabp.tile([128, SQT * HG_], BF16, tag="vslab")

            # small constants, resident for the whole kernel; loaded on the
            # scalar HWDGE queue so they never wait behind bulk traffic
            ident_sb = slabp.tile([128, 128], BF16, tag="ident")
            nc.scalar.dma_start(out=ident_sb[:], in_=ident[:])
            ones_sb = slabp.tile([1, 128], BF16, tag="onesr")
            nc.scalar.dma_start(out=ones_sb[:], in_=onesr[:])
            mask_sb = slabp.tile([128, 128], F32, tag="mask")
            nc.scalar.dma_start(out=mask_sb[:], in_=mask[:])

            # ====== era 1: q/k projections + RoPE ======
            # kt-outer "waves": 4 parallel PSUM chains (one per head) per wave,
            # weight/x DMAs interleaved per-kt so the first chain starts early,
            # rope finish deferred one wave so PE never waits on Vector/Scalar.
            with (
                tc.tile_pool(name="w1", bufs=1) as wp1,
                tc.tile_pool(name="xin1", bufs=1) as xp1,
                tc.tile_pool(name="tab", bufs=1) as tabp,
                tc.tile_pool(name="rawp", bufs=1) as rawp,
                tc.tile_pool(name="t12", bufs=1) as t12p,
                tc.tile_pool(name="ps1", bufs=6, space="PSUM") as ps1,
                tc.tile_pool(name="psrope", bufs=2, space="PSUM") as psrope,
            ):
                rT_sb = wp1.tile([128, 128], F32R, tag="rT")
                nc.scalar.dma_start(out=rT_sb[:], in_=rT[:])
                wq_sb = wp1.tile([128, KT * HG_], F32R, tag="wq")
                wk_sb = wp1.tile([128, KT * HG_], F32R, tag="wk")
                wq3 = wqT.rearrange("(kt p) j -> p kt j", p=128)
                wk3 = wkT.rearrange("(kt p) j -> p kt j", p=128)
                xT3 = xT.rearrange("(kt p) s -> p kt s", p=128)
                # weights stream on the scalar HWDGE queue; all of wq before wk
                # (q-wave consumes wq first), finer first slices for fast start
                for w_sb_, w3_, groups in (
                    (wq_sb, wq3, ((0, 2), (2, 2), (4, 4), (8, 4), (12, 4))),
                    (wk_sb, wk3, ((0, 4), (4, 4), (8, 4), (12, 4))),
                ):
                    for k0, nkt in groups:
                        nc.scalar.dma_start(
                            out=w_sb_[:, k0 * HG_:(k0 + nkt) * HG_].rearrange(
                                "p (kt j) -> p kt j", kt=nkt
                            ),
                            in_=w3_[:, k0:k0 + nkt, :],
                        )

                def flush_rope(raws, ropes, cs, cos_t, sin_t):
                    for h in range(NH_):
                        rotps = psrope.tile([128, CH], F32, tag="rope", name="rotps")
                        nc.tensor.matmul(rotps[:], rT_sb[:], raws[h][:], start=True, stop=True)
                        t1 = t12p.tile([128, CH], F32, tag="t1", name="t1")
                        nc.vector.tensor_mul(t1[:], rotps[:], sin_t[:])
                        t2 = t12p.tile([128, CH], F32, tag="t2", name="t2")
                        nc.vector.tensor_mul(t2[:], raws[h][:].bitcast(F32), cos_t[:])
                        nc.vector.tensor_add(ropes[h][:, cs], t1[:], t2[:])

                pend = None
                for sc in range(NCHUNK):
                    cs = slice(sc * CH, (sc + 1) * CH)
                    # x tiles: 2-kt-group DMAs on the sync queue (fine grain so
                    # the first wave matmul starts as early as possible)
                    xgrp = []
                    for g in range(KT // 2):
                        t = xp1.tile([128, 2 * CH], F32R, tag=f"xg{g}", name=f"xg{g}")
                        nc.sync.dma_start(
                            out=t[:].rearrange("p (kt s) -> p kt s", kt=2),
                            in_=xT3[:, g * 2:(g + 1) * 2, cs],
                        )
                        xgrp.append(t)
                    cos_t = tabp.tile([128, CH], F32, tag="cos")
                    nc.scalar.dma_start(out=cos_t[:], in_=cosT[:, cs])
                    sin_t = tabp.tile([128, CH], F32, tag="sin")
                    nc.scalar.dma_start(out=sin_t[:], in_=sinT[:, cs])

                    for w_sb, ropes in ((wq_sb, qrope), (wk_sb, krope)):
                        ps4 = [
                            ps1.tile([128, CH], F32, tag="big", name=f"ps{h}")
                            for h in range(NH_)
                        ]
                        for kt in range(KT):
                            xk_t = xgrp[kt // 2][:, (kt % 2) * CH:(kt % 2 + 1) * CH]
                            for h in range(NH_):
                                nc.tensor.matmul(
                                    ps4[h][:],
                                    w_sb[:, kt * HG_ + h * 128: kt * HG_ + (h + 1) * 128],
                                    xk_t,
                                    start=(kt == 0),
                                    stop=(kt == KT - 1),
                                )
                        # finish previous wave's rope first (its Vector ops are
                        # ready early), then drain this wave's PSUM banks
                        if pend is not None:
                            flush_rope(*pend)
                        raws = []
                        for h in range(NH_):
                            raw = rawp.tile([128, CH], F32R, tag=f"raw{h}", name=f"raw{h}")
                            nc.vector.tensor_copy(raw[:], ps4[h][:])
                            raws.append(raw)
                        pend = (raws, ropes, cs, cos_t, sin_t)
                flush_rope(*pend)

            # ====== era 2: attention with v projection interleaved ======
            with (
                tc.tile_pool(name="w2", bufs=1) as wp2,
                tc.tile_pool(name="xin2", bufs=2) as xp2,
                tc.tile_pool(name="pslab", bufs=4) as pslabp,
                tc.tile_pool(name="ptpool", bufs=2) as ptp,
                tc.tile_pool(name="ctxpool", bufs=1) as ctxp,
                tc.tile_pool(name="ostage", bufs=2) as ostp,
                tc.tile_pool(name="psbig", bufs=6, space="PSUM") as psbig,
                tc.tile_pool(name="pssmall", bufs=2, space="PSUM") as pssmall,
            ):
                # wv first (v-proj is the first consumer), 4-kt groups on the
                # scalar HWDGE queue; wo afterwards
                wv_sb = wp2.tile([128, KT * HG_], BF16, tag="wv")
                wv3 = wvT.rearrange("(kt p) j -> p kt j", p=128)
                for g in range(KT // 4):
                    nc.scalar.dma_start(
                        out=wv_sb[:, g * 4 * HG_:(g + 1) * 4 * HG_].rearrange(
                            "p (kt j) -> p kt j", kt=4
                        ),
                        in_=wv3[:, g * 4:(g + 1) * 4, :],
                    )
                wo_sb = wp2.tile([128, NH_ * H_], BF16, tag="wo")
                nc.scalar.dma_start(
                    out=wo_sb[:].rearrange("p (j ho) -> p j ho", j=NH_),
                    in_=woT.rearrange("(j p) ho -> p j ho", p=128),
                )

                xbf3 = xbfT.rearrange("(kt p) s -> p kt s", p=128)

                def vproj_tile(t):
                    xv = xp2.tile([128, KT * 128], BF16, tag="xv")
                    nc.sync.dma_start(
                        out=xv[:].rearrange("p (kt s) -> p kt s", kt=KT),
                        in_=xbf3[:, :, t * 128:(t + 1) * 128],
                    )
                    vps = psbig.tile([128, HG_], F32, tag="big")
                    for kt in range(KT):
                        nc.tensor.matmul(
                            vps[:],
                            xv[:, kt * 128:(kt + 1) * 128],
                            wv_sb[:, kt * HG_:(kt + 1) * HG_],
                            start=(kt == 0),
                            stop=(kt == KT - 1),
                        )
                    nc.scalar.copy(vslab[:, t * HG_:(t + 1) * HG_], vps[:])

                ctxT = [ctxp.tile([128, S_], BF16, tag=f"ctxT{h}", name=f"ctxT{h}") for h in range(NH_)]

                def do_transposes(ptg, pbf, sq, c):
                    # 4 PE transposes share one PSUM bank, drained by a single
                    # wide strided copy into the pt slab (fewer instructions,
                    # less per-copy overhead)
                    off = (sq - 4 * c) * 128
                    ptv = ptg[:].rearrange("p (t ch) -> p t ch", ch=CH)
                    for g in range(sq // 4 + 1):
                        gn = min(4, sq + 1 - 4 * g)
                        bank = pssmall.tile([128, CH], BF16, tag="small", name="trbank")
                        for u in range(gn):
                            nc.tensor.transpose(
                                bank[:, u * 128:(u + 1) * 128],
                                pbf[g][:, u * 128:(u + 1) * 128],
                                ident_sb[:],
                            )
                        src = bank[:].rearrange("p (t c) -> p t c", c=128)[:, :gn]
                        dst = ptv[:, 4 * g:4 * g + gn, off:off + 128]
                        if g % 2 == 1:
                            nc.scalar.copy(dst, src)
                        else:
                            nc.vector.tensor_copy(dst, src)

                def attn_scores(h, c):
                    """Scores + softmax + P transposes for (h, c)."""
                    rcp4 = statp.tile([128, 4], BF16, tag=f"rcp4_{h % 2}")
                    ptg = ptp.tile([128, SQT * CH], BF16, tag="ptslab", name="ptslab")
                    pend_tr = None
                    for sq in range(4 * c, 4 * c + 4):
                        nch = sq // 4 + 1
                        ncols = (sq + 1) * 128
                        mx = statp.tile([128, NCHUNK], F32, tag="mx")
                        scps_list = []
                        for kc in range(nch):
                            cols = min(CH, ncols - kc * CH)
                            scps = psbig.tile([128, CH], F32, tag="big")
                            nc.tensor.matmul(
                                scps[:, :cols],
                                qrope[h][:, sq * 128:(sq + 1) * 128],
                                krope[h][:, kc * CH: kc * CH + cols],
                                start=True,
                                stop=True,
                            )
                            if kc == nch - 1:
                                dcol = sq * 128 - kc * CH
                                nc.vector.tensor_add(
                                    scps[:, dcol:dcol + 128],
                                    scps[:, dcol:dcol + 128],
                                    mask_sb[:],
                                )
                            if nch > 1:
                                nc.vector.tensor_reduce(
                                    mx[:, kc:kc + 1], scps[:, :cols], axis=AX.X, op=ALU.max
                                )
                            scps_list.append((scps, cols))
                        negm = statp.tile([128, 1], F32, tag="negm")
                        if nch == 1:
                            scps0, cols0 = scps_list[0]
                            nc.vector.tensor_reduce(
                                negm[:], scps0[:, :cols0], axis=AX.X, op=ALU.max, negate=True
                            )
                        else:
                            nc.vector.tensor_reduce(
                                negm[:], mx[:, :nch], axis=AX.X, op=ALU.max, negate=True
                            )
                        # unnormalized P in bf16; row sums accumulate on ACT
                        pbf = [
                            pslabp.tile([128, CH], BF16, tag=f"pbf{kc}", name=f"pbf{kc}")
                            for kc in range(nch)
                        ]
                        ssum = statp.tile([128, NCHUNK], F32, tag="ssum")
                        for kc, (scps, cols) in enumerate(scps_list):
                            nc.scalar.activation(
                                pbf[kc][:, :cols],
                                scps[:, :cols],
                                ACTF.Exp,
                                bias=negm[:],
                                accum_out=ssum[:, kc:kc + 1],
                            )
                        rsum = statp.tile([128, 1], F32, tag="rsum")
                        nc.vector.tensor_reduce(
                            rsum[:], ssum[:, :nch], axis=AX.X, op=ALU.add
                        )
                        with nc.allow_low_precision(reason="bf16 softmax normalizer, ~0.4% rel"):
                            nc.vector.reciprocal(rcp4[:, sq - 4 * c: sq - 4 * c + 1], rsum[:])
                        if pend_tr is not None:
                            do_transposes(ptg, *pend_tr, c)
                        pend_tr = (pbf, sq)
                    do_transposes(ptg, *pend_tr, c)
                    return rcp4, ptg

                def attn_ctx(h, c, state):
                    """P^T @ V and normalization for (h, c)."""
                    rcp4, ptg = state
                    ptv = ptg[:].rearrange("p (t ch) -> p t ch", ch=CH)
                    ctxps = psbig.tile([128, CH], F32, tag="big", name="ctxps")
                    tmax = 4 * c + 4
                    for t in range(tmax):
                        c0 = max(0, (t - 4 * c) * 128)
                        nc.tensor.matmul(
                            ctxps[:, c0:CH],
                            vslab[:, t * HG_ + h * 128: t * HG_ + (h + 1) * 128],
                            ptv[:, t, c0:CH],
                            start=(t == 0),
                            stop=(t == tmax - 1),
                        )
                    # broadcast the 4 reciprocal-sum columns into a [128, CH] tile
                    rowps = pssmall.tile([1, CH], BF16, tag="small")
                    for j in range(4):
                        nc.tensor.transpose(
                            rowps[0:1, j * 128:(j + 1) * 128],
                            rcp4[:, j:j + 1],
                            ident_sb[:],
                        )
                    rrow = rowp.tile([1, CH], BF16, tag="rrow")
                    nc.scalar.copy(rrow[:], rowps[:])
                    bcps = pssmall.tile([128, CH], F32, tag="small")
                    nc.tensor.matmul(bcps[:], ones_sb[:], rrow[:], start=True, stop=True)
                    bcsb = rowp.tile([128, CH], F32, tag="bcsb")
                    nc.scalar.copy(bcsb[:], bcps[:])
                    nc.vector.tensor_mul(ctxT[h][:, c * CH:(c + 1) * CH], ctxps[:], bcsb[:])

                def out_proj(c):
                    for st in range(4 * c, 4 * c + 4):
                        ostg = ostp.tile([128, H_], F32, tag="ostg", name="ostg")
                        for hoc in range(H_ // CH):
                            wops = psbig.tile([128, CH], F32, tag="big", name="wops")
                            for j in range(NH_):
                                nc.tensor.matmul(
                                    wops[:],
                                    ctxT[j][:, st * 128:(st + 1) * 128],
                                    wo_sb[:, j * H_ + hoc * CH: j * H_ + (hoc + 1) * CH],
                                    start=(j == 0),
                                    stop=(j == NH_ - 1),
                                )
                            if c == NCHUNK - 1 and hoc % 2 == 1:
                                nc.vector.tensor_copy(ostg[:, hoc * CH:(hoc + 1) * CH], wops[:])
                            else:
                                nc.scalar.copy(ostg[:, hoc * CH:(hoc + 1) * CH], wops[:])
                            if c == NCHUNK - 1:
                                # final chunk: drain per-hoc so the last DMAs
                                # overlap the remaining copies
                                nc.sync.dma_start(
                                    out=out[st * 128:(st + 1) * 128, hoc * CH:(hoc + 1) * CH],
                                    in_=ostg[:, hoc * CH:(hoc + 1) * CH],
                                )
                        if c != NCHUNK - 1:
                            nc.sync.dma_start(out=out[st * 128:(st + 1) * 128, :], in_=ostg[:])

                # c-outer, h-inner; ctx deferred one head behind scores so the
                # v projection (and its DMAs) hide behind two heads of scores.
                # The output projection of chunk c-1 slots in after the first
                # scores of chunk c, covering that block's softmax latency.
                for c in range(NCHUNK):
                    r0 = attn_scores(0, c)
                    if c > 0:
                        out_proj(c - 1)
                    r1 = attn_scores(1, c)
                    for t in range(4 * c, 4 * c + 4):
                        vproj_tile(t)
                    attn_ctx(0, c, r0)
                    r2 = attn_scores(2, c)
                    attn_ctx(1, c, r1)
                    r3 = attn_scores(3, c)
                    attn_ctx(2, c, r2)
                    attn_ctx(3, c, r3)
                out_proj(NCHUNK - 1)




    nc.compile()
    return nc


def _make_tables(S_, D_=128):
    inv_freq = 1.0 / (ROPE_BASE ** (np.arange(0, D_, 2, dtype=np.float32) / D_))
    pos = np.arange(S_, dtype=np.float32)
    ang = pos[:, None] * inv_freq[None, :]
    ang = np.concatenate([ang, ang], axis=1)
    return (
        np.cos(ang).T.astype(np.float32).copy(),
        np.sin(ang).T.astype(np.float32).copy(),
    )


def _make_rot_T(D_=128):
    R = np.zeros((D_, D_), dtype=np.float32)
    half = D_ // 2
    for d in range(half):
        R[d, d + half] = -1.0
    for d in range(half, D_):
        R[d, d - half] = 1.0
    return R.T.copy()


def _make_mask(mask_val=-1e30):
    m = np.zeros((128, 128), dtype=np.float32)
    m[np.triu_indices(128, k=1)] = mask_val
    return m


def kernel(x, Wq, Wk, Wv, Wo):
    """Full inputs in, full output out. Shards over 8 NeuronCores internally."""
    global LAST_RESULTS
    x = np.ascontiguousarray(np.asarray(x, dtype=np.float32))
    Wq = np.asarray(Wq, dtype=np.float32)
    Wk = np.asarray(Wk, dtype=np.float32)
    Wv = np.asarray(Wv, dtype=np.float32)
    Wo = np.asarray(Wo, dtype=np.float32)

    if "nc" not in _NC_CACHE:
        _NC_CACHE["nc"] = _build()
    nc = _NC_CACHE["nc"]

    scale = np.sqrt(np.float32(D))
    cosT, sinT = _make_tables(S)
    rT = _make_rot_T()
    identb = np.eye(128, dtype=ml_dtypes.bfloat16)
    identf = np.eye(128, dtype=np.float32)
    onesr = np.ones((1, 128), dtype=ml_dtypes.bfloat16)
    maskt = _make_mask()

    WqT = Wq.T * scale                    # [H, 16*D], scale folded into q path
    WkT = np.ascontiguousarray(Wk.T)
    WvT_bf = Wv.T.astype(ml_dtypes.bfloat16)
    WoT_bf = Wo.T.astype(ml_dtypes.bfloat16)   # [H(in=ctx), H(out)] rows = ctx hidden

    in_maps = []
    for c in range(N_CORES):
        b, g = divmod(c, NH)
        js = slice(g * HG, (g + 1) * HG)
        xT_b = np.ascontiguousarray(x[b].T)
        in_maps.append({
            "xT": xT_b,
            "xbfT": xT_b.astype(ml_dtypes.bfloat16),
            "wqT": np.ascontiguousarray(WqT[:, js]).astype(np.float32),
            "wkT": np.ascontiguousarray(WkT[:, js]),
            "wvT": np.ascontiguousarray(WvT_bf[:, js]),
            "woT": np.ascontiguousarray(WoT_bf[js, :]),
            "cosT": cosT,
            "sinT": sinT,
            "rT": rT,
            "ident": identb,
            "identf": identf,
            "onesr": onesr,
            "mask": maskt,
        })

    LAST_RESULTS = run_bass_kernel_spmd(
        nc, in_maps, core_ids=list(range(N_CORES)), trace=TRACE
    )
    res = LAST_RESULTS.results

    out = np.zeros((B, S, H), dtype=np.float32)
    for c in range(N_CORES):
        b = c // NH
        out[b] += res[c]["out"]
    return out



# revision 3
# speedup vs baseline: 1.1639x; 1.1639x over previous
"""Self-contained Trainium2 kernel for nn_AutoregressiveGroupQuerySelfAttention.

Reference computation (B=2, S=2048, H=2048, 16 heads x 128 dim):
    q = (x @ Wq.T) -> heads; k likewise; v likewise
    q, k get RoPE; scores = (q @ k.T) * sqrt(D)   (faithful-to-source bug)
    causal softmax; ctx = attn @ v; out = ctx @ Wo.T

Sharding over 8 NeuronCores: core c = (b, g) with b = c // 4 (batch),
g = c % 4 (head-group of 4 heads = 512 hidden columns).  Each core computes
its head-group's context and a partial output  ctx_g @ Wo.T[g-rows, :];
the host sums the 4 partials per batch element.

Precision: the softmax is nearly one-hot (the sqrt(D) score scaling makes
logits ~N(0,128^2)), so the logit path (q/k projections + scores) runs in
fp32r (full-speed reduced-precision fp32 matmul, ~1.5e-4 rel err); v/P/Wo
run in bf16.  Measured end-to-end rel err vs the fp32 reference ~5e-3.

Schedule (per core):
  era 1 - q/k projections as kt-outer "waves" of 4 parallel PSUM chains
  (one per head), weights/x streamed in groups on the two HWDGE queues,
  RoPE finish deferred one wave so the PE never waits on Vector/Scalar.
  era 2 - c-outer/h-inner attention: per 512-row chunk, scores+softmax of
  heads 0-1 cover the v-projection DMAs, each head's P^T@V is deferred one
  head behind its scores, P transposes drain 4-at-a-time from a shared
  PSUM bank via one wide strided copy, and the output projection of chunk
  c-1 executes inside chunk c's first scores block to hide softmax latency.
"""
import numpy as np
import ml_dtypes

import concourse.bass as bass
import concourse.mybir as mybir
from concourse import bacc
from concourse.tile import TileContext
from concourse.bass_utils import run_bass_kernel_spmd

F32 = mybir.dt.float32
F32R = mybir.dt.float32r
BF16 = mybir.dt.bfloat16
AX = mybir.AxisListType
ALU = mybir.AluOpType
ACTF = mybir.ActivationFunctionType

B, S, H = 2, 2048, 2048
NUM_HEADS, D = 16, 128
N_CORES = 8
NH = 4                     # heads per core
HG = NH * D                # 512
ROPE_BASE = 10000.0

_NC_CACHE = {}
LAST_RESULTS = None        # BassKernelResults of the most recent run (for profiling)
TRACE = False


def _build(S_=S, H_=H, NH_=NH):
    DD = 128
    HG_ = NH_ * DD
    KT = H_ // 128
    SQT = S_ // 128
    CH = 512
    NCHUNK = S_ // CH

    nc = bacc.Bacc()
    xT = nc.declare_dram_parameter("xT", [H_, S_], F32R, isOutput=False)
    xbfT = nc.declare_dram_parameter("xbfT", [H_, S_], BF16, isOutput=False)
    wqT = nc.declare_dram_parameter("wqT", [H_, HG_], F32R, isOutput=False)
    wkT = nc.declare_dram_parameter("wkT", [H_, HG_], F32R, isOutput=False)
    wvT = nc.declare_dram_parameter("wvT", [H_, HG_], BF16, isOutput=False)
    woT = nc.declare_dram_parameter("woT", [HG_, H_], BF16, isOutput=False)
    cosT = nc.declare_dram_parameter("cosT", [128, S_], F32, isOutput=False)
    sinT = nc.declare_dram_parameter("sinT", [128, S_], F32, isOutput=False)
    rT = nc.declare_dram_parameter("rT", [128, 128], F32R, isOutput=False)
    ident = nc.declare_dram_parameter("ident", [128, 128], BF16, isOutput=False)
    identf = nc.declare_dram_parameter("identf", [128, 128], F32, isOutput=False)
    onesr = nc.declare_dram_parameter("onesr", [1, 128], BF16, isOutput=False)
    mask = nc.declare_dram_parameter("mask", [128, 128], F32, isOutput=False)
    out = nc.declare_dram_parameter("out", [S_, H_], F32, isOutput=True)

    with TileContext(nc) as tc:
        with (
            tc.tile_pool(name="slabs", bufs=1) as slabp,
            tc.tile_pool(name="stats", bufs=3) as statp,
            tc.tile_pool(name="rows", bufs=2) as rowp,
        ):
            qrope = [slabp.tile([128, S_], F32R, tag=f"qrope{h}", name=f"qrope{h}") for h in range(NH_)]
            krope = [slabp.tile([128, S_], F32R, tag=f"krope{h}", name=f"krope{h}") for h in range(NH_)]
            vslab = slabp.tile([128, SQT * HG_], BF16, tag="vslab")

            # small constants, resident for the whole kernel; loaded on the
            # scalar HWDGE queue so they never wait behind bulk traffic
            ident_sb = slabp.tile([128, 128], BF16, tag="ident")
            nc.scalar.dma_start(out=ident_sb[:], in_=ident[:])
            ones_sb = slabp.tile([1, 128], BF16, tag="onesr")
            nc.scalar.dma_start(out=ones_sb[:], in_=onesr[:])
            mask_sb = slabp.tile([128, 128], F32, tag="mask")
            nc.scalar.dma_start(out=mask_sb[:], in_=mask[:])

            # ====== era 1: q/k projections + RoPE ======
            # kt-outer "waves": 4 parallel PSUM chains (one per head) per wave,
            # weight/x DMAs interleaved per-kt so the first chain starts early,
            # rope finish deferred one wave so PE never waits on Vector/Scalar.
            with (
                tc.tile_pool(name="w1", bufs=1) as wp1,
                tc.tile_pool(name="xin1", bufs=1) as xp1,
                tc.tile_pool(name="tab", bufs=1) as tabp,
                tc.tile_pool(name="rawp", bufs=1) as rawp,
                tc.tile_pool(name="t12", bufs=1) as t12p,
                tc.tile_pool(name="ps1", bufs=6, space="PSUM") as ps1,
                tc.tile_pool(name="psrope", bufs=2, space="PSUM") as psrope,
            ):
                rT_sb = wp1.tile([128, 128], F32R, tag="rT")
                nc.scalar.dma_start(out=rT_sb[:], in_=rT[:])
                wq_sb = wp1.tile([128, KT * HG_], F32R, tag="wq")
                wk_sb = wp1.tile([128, KT * HG_], F32R, tag="wk")
                wq3 = wqT.rearrange("(kt p) j -> p kt j", p=128)
                wk3 = wkT.rearrange("(kt p) j -> p kt j", p=128)
                xT3 = xT.rearrange("(kt p) s -> p kt s", p=128)
                # weights stream on the scalar HWDGE queue; all of wq before wk
                # (q-wave consumes wq first), finer first slices for fast start
                for w_sb_, w3_, groups in (
                    (wq_sb, wq3, ((0, 2), (2, 2), (4, 4), (8, 4), (12, 4))),
                    (wk_sb, wk3, ((0, 4), (4, 4), (8, 4), (12, 4))),
                ):
                    for k0, nkt in groups:
                        nc.scalar.dma_start(
                            out=w_sb_[:, k0 * HG_:(k0 + nkt) * HG_].rearrange(
                                "p (kt j) -> p kt j", kt=nkt
                            ),
                            in_=w3_[:, k0:k0 + nkt, :],
                        )

                def flush_rope(raws, ropes, cs, cos_t, sin_t):
                    for h in range(NH_):
                        rotps = psrope.tile([128, CH], F32, tag="rope", name="rotps")
                        nc.tensor.matmul(rotps[:], rT_sb[:], raws[h][:], start=True, stop=True)
                        t1 = t12p.tile([128, CH], F32, tag="t1", name="t1")
                        nc.vector.tensor_mul(t1[:], rotps[:], sin_t[:])
                        t2 = t12p.tile([128, CH], F32, tag="t2", name="t2")
                        nc.vector.tensor_mul(t2[:], raws[h][:].bitcast(F32), cos_t[:])
                        nc.vector.tensor_add(ropes[h][:, cs], t1[:], t2[:])

                pend = None
                for sc in range(NCHUNK):
                  with nc.named_scope(f"e1c{sc}"):
                    cs = slice(sc * CH, (sc + 1) * CH)
                    # x tiles: 2-kt-group DMAs on the sync queue (fine grain so
                    # the first wave matmul starts as early as possible)
                    xgrp = []
                    for g in range(KT // 2):
                        t = xp1.tile([128, 2 * CH], F32R, tag=f"xg{g}", name=f"xg{g}")
                        nc.sync.dma_start(
                            out=t[:].rearrange("p (kt s) -> p kt s", kt=2),
                            in_=xT3[:, g * 2:(g + 1) * 2, cs],
                        )
                        xgrp.append(t)
                    cos_t = tabp.tile([128, CH], F32, tag="cos")
                    nc.scalar.dma_start(out=cos_t[:], in_=cosT[:, cs])
                    sin_t = tabp.tile([128, CH], F32, tag="sin")
                    nc.scalar.dma_start(out=sin_t[:], in_=sinT[:, cs])

                    for w_sb, ropes in ((wq_sb, qrope), (wk_sb, krope)):
                        ps4 = [
                            ps1.tile([128, CH], F32, tag="big", name=f"ps{h}")
                            for h in range(NH_)
                        ]
                        for kt in range(KT):
                            xk_t = xgrp[kt // 2][:, (kt % 2) * CH:(kt % 2 + 1) * CH]
                            for h in range(NH_):
                                nc.tensor.matmul(
                                    ps4[h][:],
                                    w_sb[:, kt * HG_ + h * 128: kt * HG_ + (h + 1) * 128],
                                    xk_t,
                                    start=(kt == 0),
                                    stop=(kt == KT - 1),
                                )
                        # finish previous wave's rope first (its Vector ops are
                        # ready early), then drain this wave's PSUM banks
                        if pend is not None:
                            flush_rope(*pend)
                        raws = []
                        for h in range(NH_):
                            raw = rawp.tile([128, CH], F32R, tag=f"raw{h}", name=f"raw{h}")
                            nc.vector.tensor_copy(raw[:], ps4[h][:])
                            raws.append(raw)
                        pend = (raws, ropes, cs, cos_t, sin_t)
                flush_rope(*pend)

            # ====== era 2: attention with v projection interleaved ======
            with (
                tc.tile_pool(name="w2", bufs=1) as wp2,
                tc.tile_pool(name="xin2", bufs=2) as xp2,
                tc.tile_pool(name="pslab", bufs=4) as pslabp,
                tc.tile_pool(name="ptpool", bufs=2) as ptp,
                tc.tile_pool(name="ctxpool", bufs=1) as ctxp,
                tc.tile_pool(name="ostage", bufs=2) as ostp,
                tc.tile_pool(name="psbig", bufs=6, space="PSUM") as psbig,
                tc.tile_pool(name="pssmall", bufs=2, space="PSUM") as pssmall,
            ):
                # wv first (v-proj is the first consumer), 4-kt groups on the
                # scalar HWDGE queue; wo afterwards
                wv_sb = wp2.tile([128, KT * HG_], BF16, tag="wv")
                wv3 = wvT.rearrange("(kt p) j -> p kt j", p=128)
                for g in range(KT // 4):
                    nc.scalar.dma_start(
                        out=wv_sb[:, g * 4 * HG_:(g + 1) * 4 * HG_].rearrange(
                            "p (kt j) -> p kt j", kt=4
                        ),
                        in_=wv3[:, g * 4:(g + 1) * 4, :],
                    )
                wo_sb = wp2.tile([128, NH_ * H_], BF16, tag="wo")
                nc.scalar.dma_start(
                    out=wo_sb[:].rearrange("p (j ho) -> p j ho", j=NH_),
                    in_=woT.rearrange("(j p) ho -> p j ho", p=128),
                )

                xbf3 = xbfT.rearrange("(kt p) s -> p kt s", p=128)

                def vproj_tile(t):
                    xv = xp2.tile([128, KT * 128], BF16, tag="xv")
                    nc.sync.dma_start(
                        out=xv[:].rearrange("p (kt s) -> p kt s", kt=KT),
                        in_=xbf3[:, :, t * 128:(t + 1) * 128],
                    )
                    vps = psbig.tile([128, HG_], F32, tag="big")
                    for kt in range(KT):
                        nc.tensor.matmul(
                            vps[:],
                            xv[:, kt * 128:(kt + 1) * 128],
                            wv_sb[:, kt * HG_:(kt + 1) * HG_],
                            start=(kt == 0),
                            stop=(kt == KT - 1),
                        )
                    nc.scalar.copy(vslab[:, t * HG_:(t + 1) * HG_], vps[:])

                ctxT = [ctxp.tile([128, S_], BF16, tag=f"ctxT{h}", name=f"ctxT{h}") for h in range(NH_)]

                def do_transposes(ptg, pbf, sq, c):
                    # 4 PE transposes share one PSUM bank, drained by a single
                    # wide strided copy into the pt slab (fewer instructions,
                    # less per-copy overhead)
                    off = (sq - 4 * c) * 128
                    ptv = ptg[:].rearrange("p (t ch) -> p t ch", ch=CH)
                    for g in range(sq // 4 + 1):
                        gn = min(4, sq + 1 - 4 * g)
                        bank = pssmall.tile([128, CH], BF16, tag="small", name="trbank")
                        for u in range(gn):
                            nc.tensor.transpose(
                                bank[:, u * 128:(u + 1) * 128],
                                pbf[g][:, u * 128:(u + 1) * 128],
                                ident_sb[:],
                            )
                        src = bank[:].rearrange("p (t c) -> p t c", c=128)[:, :gn]
                        dst = ptv[:, 4 * g:4 * g + gn, off:off + 128]
                        if g % 2 == 1:
                            nc.scalar.copy(dst, src)
                        else:
                            nc.vector.tensor_copy(dst, src)

                def attn_scores(h, c):
                    """Scores + softmax + P transposes for (h, c)."""
                    rcp4 = statp.tile([128, 4], BF16, tag=f"rcp4_{h % 2}")
                    ptg = ptp.tile([128, SQT * CH], BF16, tag="ptslab", name="ptslab")
                    pend_tr = None
                    for sq in range(4 * c, 4 * c + 4):
                        nch = sq // 4 + 1
                        ncols = (sq + 1) * 128
                        mx = statp.tile([128, NCHUNK], F32, tag="mx")
                        scps_list = []
                        for kc in range(nch):
                            cols = min(CH, ncols - kc * CH)
                            scps = psbig.tile([128, CH], F32, tag="big")
                            nc.tensor.matmul(
                                scps[:, :cols],
                                qrope[h][:, sq * 128:(sq + 1) * 128],
                                krope[h][:, kc * CH: kc * CH + cols],
                                start=True,
                                stop=True,
                            )
                            if kc == nch - 1:
                                dcol = sq * 128 - kc * CH
                                nc.vector.tensor_add(
                                    scps[:, dcol:dcol + 128],
                                    scps[:, dcol:dcol + 128],
                                    mask_sb[:],
                                )
                            if nch > 1:
                                nc.vector.tensor_reduce(
                                    mx[:, kc:kc + 1], scps[:, :cols], axis=AX.X, op=ALU.max
                                )
                            scps_list.append((scps, cols))
                        negm = statp.tile([128, 1], F32, tag="negm")
                        if nch == 1:
                            scps0, cols0 = scps_list[0]
                            nc.vector.tensor_reduce(
                                negm[:], scps0[:, :cols0], axis=AX.X, op=ALU.max, negate=True
                            )
                        else:
                            nc.vector.tensor_reduce(
                                negm[:], mx[:, :nch], axis=AX.X, op=ALU.max, negate=True
                            )
                        # unnormalized P in bf16; row sums accumulate on ACT
                        pbf = [
                            pslabp.tile([128, CH], BF16, tag=f"pbf{kc}", name=f"pbf{kc}")
                            for kc in range(nch)
                        ]
                        ssum = statp.tile([128, NCHUNK], F32, tag="ssum")
                        for kc, (scps, cols) in enumerate(scps_list):
                            nc.scalar.activation(
                                pbf[kc][:, :cols],
                                scps[:, :cols],
                                ACTF.Exp,
                                bias=negm[:],
                                accum_out=ssum[:, kc:kc + 1],
                            )
                        rsum = statp.tile([128, 1], F32, tag="rsum")
                        nc.vector.tensor_reduce(
                            rsum[:], ssum[:, :nch], axis=AX.X, op=ALU.add
                        )
                        with nc.allow_low_precision(reason="bf16 softmax normalizer, ~0.4% rel"):
                            nc.vector.reciprocal(rcp4[:, sq - 4 * c: sq - 4 * c + 1], rsum[:])
                        if pend_tr is not None:
                            do_transposes(ptg, *pend_tr, c)
                        pend_tr = (pbf, sq)
                    do_transposes(ptg, *pend_tr, c)
                    return rcp4, ptg

                def attn_ctx(h, c, state):
                    """P^T @ V and normalization for (h, c)."""
                    rcp4, ptg = state
                    ptv = ptg[:].rearrange("p (t ch) -> p t ch", ch=CH)
                    ctxps = psbig.tile([128, CH], F32, tag="big", name="ctxps")
                    tmax = 4 * c + 4
                    for t in range(tmax):
                        c0 = max(0, (t - 4 * c) * 128)
                        nc.tensor.matmul(
                            ctxps[:, c0:CH],
                            vslab[:, t * HG_ + h * 128: t * HG_ + (h + 1) * 128],
                            ptv[:, t, c0:CH],
                            start=(t == 0),
                            stop=(t == tmax - 1),
                        )
                    # broadcast the 4 reciprocal-sum columns into a [128, CH] tile
                    rowps = pssmall.tile([1, CH], BF16, tag="small")
                    for j in range(4):
                        nc.tensor.transpose(
                            rowps[0:1, j * 128:(j + 1) * 128],
                            rcp4[:, j:j + 1],
                            ident_sb[:],
                        )
                    rrow = rowp.tile([1, CH], BF16, tag="rrow")
                    nc.scalar.copy(rrow[:], rowps[:])
                    bcps = pssmall.tile([128, CH], F32, tag="small")
                    nc.tensor.matmul(bcps[:], ones_sb[:], rrow[:], start=True, stop=True)
                    bcsb = rowp.tile([128, CH], F32, tag="bcsb")
                    nc.scalar.copy(bcsb[:], bcps[:])
                    nc.vector.tensor_mul(ctxT[h][:, c * CH:(c + 1) * CH], ctxps[:], bcsb[:])

                def out_proj(c):
                    for st in range(4 * c, 4 * c + 4):
                        ostg = ostp.tile([128, H_], F32, tag="ostg", name="ostg")
                        for hoc in range(H_ // CH):
                            wops = psbig.tile([128, CH], F32, tag="big", name="wops")
                            for j in range(NH_):
                                nc.tensor.matmul(
                                    wops[:],
                                    ctxT[j][:, st * 128:(st + 1) * 128],
                                    wo_sb[:, j * H_ + hoc * CH: j * H_ + (hoc + 1) * CH],
                                    start=(j == 0),
                                    stop=(j == NH_ - 1),
                                )
                            if c == NCHUNK - 1 and hoc % 2 == 1:
                                nc.vector.tensor_copy(ostg[:, hoc * CH:(hoc + 1) * CH], wops[:])
                            else:
                                nc.scalar.copy(ostg[:, hoc * CH:(hoc + 1) * CH], wops[:])
                            if c == NCHUNK - 1:
                                # final chunk: drain per-hoc so the last DMAs
                                # overlap the remaining copies
                                nc.sync.dma_start(
                                    out=out[st * 128:(st + 1) * 128, hoc * CH:(hoc + 1) * CH],
                                    in_=ostg[:, hoc * CH:(hoc + 1) * CH],
                                )
                        if c != NCHUNK - 1:
                            nc.sync.dma_start(out=out[st * 128:(st + 1) * 128, :], in_=ostg[:])

                # c-outer, h-inner; ctx deferred one head behind scores so the
                # v projection (and its DMAs) hide behind two heads of scores.
                # The output projection of chunk c-1 slots in after the first
                # scores of chunk c, covering that block's softmax latency.
                def scoped(name, f, *a):
                    with nc.named_scope(name):
                        return f(*a)

                for c in range(NCHUNK):
                    r0 = scoped(f"s{c}h0", attn_scores, 0, c)
                    if c > 0:
                        scoped(f"o{c-1}", out_proj, c - 1)
                    r1 = scoped(f"s{c}h1", attn_scores, 1, c)
                    for t in range(4 * c, 4 * c + 4):
                        scoped(f"v{c}", vproj_tile, t)
                    scoped(f"x{c}h0", attn_ctx, 0, c, r0)
                    r2 = scoped(f"s{c}h2", attn_scores, 2, c)
                    scoped(f"x{c}h1", attn_ctx, 1, c, r1)
                    r3 = scoped(f"s{c}h3", attn_scores, 3, c)
                    scoped(f"x{c}h2", attn_ctx, 2, c, r2)
                    scoped(f"x{c}h3", attn_ctx, 3, c, r3)
                scoped(f"o{NCHUNK-1}", out_proj, NCHUNK - 1)




    nc.compile()
    return nc


def _make_tables(S_, D_=128):
    inv_freq = 1.0 / (ROPE_BASE ** (np.arange(0, D_, 2, dtype=np.float32) / D_))
    pos = np.arange(S_, dtype=np.float32)
    ang = pos[:, None] * inv_freq[None, :]
    ang = np.concatenate([ang, ang], axis=1)
    return (
        np.cos(ang).T.astype(np.float32).copy(),
        np.sin(ang).T.astype(np.float32).copy(),
    )


def _make_rot_T(D_=128):
    R = np.zeros((D_, D_), dtype=np.float32)
    half = D_ // 2
    for d in range(half):
        R[d, d + half] = -1.0
    for d in range(half, D_):
        R[d, d - half] = 1.0
    return R.T.copy()


def _make_mask(mask_val=-1e30):
    m = np.zeros((128, 128), dtype=np.float32)
    m[np.triu_indices(128, k=1)] = mask_val
    return m


def kernel(x, Wq, Wk, Wv, Wo):
    """Full inputs in, full output out. Shards over 8 NeuronCores internally."""
    global LAST_RESULTS
    x = np.ascontiguousarray(np.asarray(x, dtype=np.float32))
    Wq = np.asarray(Wq, dtype=np.float32)
    Wk = np.asarray(Wk, dtype=np.float32)
    Wv = np.asarray(Wv, dtype=np.float32)
    Wo = np.asarray(Wo, dtype=np.float32)

    if "nc" not in _NC_CACHE:
        _NC_CACHE["nc"] = _build()
    nc = _NC_CACHE["nc"]

    scale = np.sqrt(np.float32(D))
    cosT, sinT = _make_tables(S)
    rT = _make_rot_T()
    identb = np.eye(128, dtype=ml_dtypes.bfloat16)
    identf = np.eye(128, dtype=np.float32)
    onesr = np.ones((1, 128), dtype=ml_dtypes.bfloat16)
    maskt = _make_mask()

    WqT = Wq.T * scale                    # [H, 16*D], scale folded into q path
    WkT = np.ascontiguousarray(Wk.T)
    WvT_bf = Wv.T.astype(ml_dtypes.bfloat16)
    WoT_bf = Wo.T.astype(ml_dtypes.bfloat16)   # [H(in=ctx), H(out)] rows = ctx hidden

    in_maps = []
    for c in range(N_CORES):
        b, g = divmod(c, NH)
        js = slice(g * HG, (g + 1) * HG)
        xT_b = np.ascontiguousarray(x[b].T)
        in_maps.append({
            "xT": xT_b,
            "xbfT": xT_b.astype(ml_dtypes.bfloat16),
            "wqT": np.ascontiguousarray(WqT[:, js]).astype(np.float32),
            "wkT": np.ascontiguousarray(WkT[:, js]),
            "wvT": np.ascontiguousarray(WvT_bf[:, js]),
            "woT": np.ascontiguousarray(WoT_bf[js, :]),
            "cosT": cosT,
            "sinT": sinT,
            "rT": rT,
            "ident": identb,
            "identf": identf,
            "onesr": onesr,
            "mask": maskt,
        })

    LAST_RESULTS = run_bass_kernel_spmd(
        nc, in_maps, core_ids=list(range(N_CORES)), trace=TRACE
    )
    res = LAST_RESULTS.results

    out = np.zeros((B, S, H), dtype=np.float32)
    for c in range(N_CORES):
        b = c // NH
        out[b] += res[c]["out"]
    return out

